# revision 1
# baseline (speedup 1.0000x reference)
"""Trainium2 Bass kernel for GCFM sparse-attention module.

Sharding: 8 cores = 2 batches x 4 row-blocks (12 rows each).
Each core gets reflect-padded input slabs (18 rows x 54 cols) so the
7x7 local-attention window needs no boundary logic on device; the
conv branch's zero-padding is handled with a 0/1 validity mask.

Math notes:
  - att logits = (1/8) * q . unfold(k - pe); the q.pe term is constant
    over the window axis and cancels in softmax.
  - softmax denominator comes from a constant 2.0 column in the AV rhs,
    so 1/psum[:,64] directly equals the needed 0.5/sum(exp) scale.
  - conv branch: f = Wfc x qkv (grouped over 64 g) then 3x3 depthwise,
    both as block-diagonal matmuls over 13-g blocks (0.5 folded in).
  - QK runs chunk-major: one matmul per (head, 2-window-row chunk)
    covering all row-blocks that use the chunk (N up to 384).
  - Big lhsT operands are padded to 128 columns to trigger FWL.
"""

import os
import numpy as np

# ---- hardcoded problem geometry ----
B, C, H, W = 2, 256, 48, 48
HEAD, D = 4, 64
KATT = 7
PAD = KATT // 2          # 3
NCORES = 8
RPC = H // 4             # 12 rows per core
SLAB_R, SLAB_C = RPC + 2 * PAD, W + 2 * PAD   # 18, 54
PX = SLAB_R * SLAB_C     # 972
PXP = PX + 32            # padded slab stride so 128-wide chunk reads fit
CGR, CGC = RPC + 2, W + 2   # conv grid 14 x 50
CPX = CGR * CGC          # 700
GBS = 13                 # g-block size for block-diagonal matmuls
NGB = 5                  # number of g blocks (5*13 = 65 >= 64)
MF = GBS * 9             # 117: valid M of f block
MD = GBS * 4             # 52: valid M of depthwise block
GSIZE6 = [13, 13, 6, 13, 13, 6]
GSTART6 = [0, 13, 26, 32, 45, 58]

# fp16 blob layout (per-partition column offsets)
O_XS = 0
O_YS = O_XS + 2 * PXP
O_ZS = O_YS + 2 * PXP
O_W1 = O_ZS + 2 * PXP
O_W2 = O_W1 + 512
O_W3 = O_W2 + 512
O_POS = O_W3 + 512
O_WPT = O_POS + PX
O_MAR = O_WPT + 64
O_MCS = O_MAR + 384
O_BDF = O_MCS + CPX
O_BDD = O_BDF + 36 * 128
O_ID = O_BDD + 54 * 128
NB16 = O_ID + 96

# fp32 blob (biases)
O_B1 = 0
O_B2 = 2
O_B3 = 4
O_BP = 6
O_B3BC = 7
NB32 = O_B3BC + 256

_CACHE = {}
last_results = None      # BassKernelResults from the most recent run


def _build_nc():
    """Build the (shared SPMD) Bass program once."""
    import concourse.bacc as bacc
    import concourse.mybir as mybir
    from concourse.tile import TileContext
    from contextlib import ExitStack

    F32 = mybir.dt.float32
    F16 = mybir.dt.float16
    AF = mybir.ActivationFunctionType

    nc = bacc.Bacc(None, target_bir_lowering=False)

    blob16_d = nc.declare_dram_parameter('blob16', [128, NB16], F16, isOutput=False)
    blob32_d = nc.declare_dram_parameter('blob32', [128, NB32], F32, isOutput=False)
    out_d = nc.declare_dram_parameter('out', [2, 128, RPC, W], F32, isOutput=True)

    with TileContext(nc) as tc, ExitStack() as ctx:
        P = ctx.enter_context(tc.tile_pool(name='persist', bufs=1))
        WK = ctx.enter_context(tc.tile_pool(name='work', bufs=3))
        AE = ctx.enter_context(tc.tile_pool(name='attE', bufs=18))
        PS = ctx.enter_context(tc.tile_pool(name='ps', bufs=8, space='PSUM'))

        # ---- persistent SBUF tensors ----
        b16 = P.tile([128, NB16], F16, tag='b16')
        b32 = P.tile([128, NB32], F32, tag='b32')

        qs = P.tile([128, 2, SLAB_R, SLAB_C], F16, tag='qs')
        ks = P.tile([128, 2, SLAB_R, SLAB_C], F16, tag='ks')
        vs = P.tile([128, 2, SLAB_R, SLAB_C], F16, tag='vs')
        pes = P.tile([128, SLAB_R, SLAB_C], F16, tag='pes')
        kms = P.tile([128, 2, PXP], F16, tag='kms')
        vts = P.tile([108, 4, 9, 65], F16, tag='vts')
        opx = P.tile([96, 6, 256], F16, tag='opx')
        qc = P.tile([128, 2, CGR, CGC], F16, tag='qc')
        kc = P.tile([128, 2, CGR, CGC], F16, tag='kc')
        vc = P.tile([128, 2, CGR, CGC], F16, tag='vc')
        fs = P.tile([128, 6, CGR, CGC], F16, tag='fs')
        ocv = P.tile([128, 2, RPC, W], F16, tag='ocv')
        osb = P.tile([128, 2, RPC, W], F32, tag='osb')

        # blob views
        def v16(off, n, p=128):
            return b16[0:p, off:off + n]

        def slab_v(base, t):                       # [128, 18, 54] spatial view
            return v16(base + PXP * t, PX).rearrange('p (r c) -> p r c', c=SLAB_C)

        wv = {1: O_W1, 2: O_W2, 3: O_W3}
        mcs = v16(O_MCS, CPX).rearrange('p (r c) -> p r c', c=CGC)
        id96 = v16(O_ID, 96, 96)
        posv = v16(O_POS, PX, 2).rearrange('p (r c) -> p r c', c=SLAB_C)
        wptv = v16(O_WPT, 64, 2)
        bdf_v = v16(O_BDF, 36 * 128).rearrange('p (i m) -> p i m', m=128)
        bdd_v = v16(O_BDD, 54 * 128, MF).rearrange('p (i m) -> p i m', m=128)
        b3bc = b32[0:108, O_B3BC:O_B3BC + 256]
        bpv = b32[0:64, O_BP:O_BP + 1]

        # ---- input DMAs ----
        nc.sync.dma_start(out=b16[:, O_W1:O_BDF], in_=blob16_d[:, O_W1:O_BDF])
        nc.sync.dma_start(out=b32[:], in_=blob32_d[:])
        nc.sync.dma_start(out=b16[:, O_XS:O_YS], in_=blob16_d[:, O_XS:O_YS])
        nc.sync.dma_start(out=b16[:, O_YS:O_ZS], in_=blob16_d[:, O_YS:O_ZS])
        nc.sync.dma_start(out=b16[:, O_ZS:O_W1], in_=blob16_d[:, O_ZS:O_W1])
        nc.sync.dma_start(out=b16[:, O_BDF:NB16], in_=blob16_d[:, O_BDF:NB16])

        # ---- stage 1: 1x1 convs q,k,v (ch-major) ----
        for (xo, wi, bo, dst) in ((O_XS, 1, O_B1, qs), (O_YS, 2, O_B2, ks),
                                  (O_ZS, 3, O_B3, vs)):
            for mt in range(2):
                for chk in range(2):
                    pq = PS.tile([128, 9, SLAB_C], F32, tag='ps')
                    for kt in range(2):
                        nc.tensor.matmul(
                            pq[:],
                            lhsT=v16(wv[wi] + 256 * kt + 128 * mt, 128),
                            rhs=slab_v(xo, kt)[:, 9 * chk:9 * chk + 9, :],
                            start=(kt == 0), stop=(kt == 1))
                    nc.vector.tensor_scalar_add(
                        dst[:, mt, 9 * chk:9 * chk + 9, :], pq[:],
                        b32[:, bo + mt:bo + mt + 1])

        # position encoding pe (64 ch) -> stacked twice into 128 partitions
        for chk in range(2):
            ppe = PS.tile([64, 9, SLAB_C], F32, tag='ps')
            nc.tensor.matmul(ppe[:], lhsT=wptv,
                             rhs=posv[:, 9 * chk:9 * chk + 9, :],
                             start=True, stop=True)
            nc.scalar.activation(pes[0:64, 9 * chk:9 * chk + 9, :], ppe[:],
                                 AF.Identity, bias=bpv)
            nc.scalar.activation(pes[64:128, 9 * chk:9 * chk + 9, :], ppe[:],
                                 AF.Identity, bias=bpv)

        # km = k - pe  (into FWL-padded layout; zero the 32-col tail)
        kms_sp = kms[:, :, 0:PX].rearrange('p t (r c) -> p t r c', c=SLAB_C)
        for t in range(2):
            nc.vector.tensor_sub(kms_sp[:, t], ks[:, t], pes[:])
            nc.vector.memset(kms[:, t, PX:PXP], 0.0)

        # ---- stage 2: vT (pixel-major v) via transposed conv from z ----
        nc.vector.memset(vts[:, :, :, 64], 2.0)
        for pb in range(9):
            pvt = PS.tile([128, 256], F32, tag='ps')
            for kt in range(2):
                nc.tensor.matmul(
                    pvt[:],
                    lhsT=v16(O_ZS + PXP * kt + 108 * pb, 128),
                    rhs=v16(wv[3] + 256 * kt, 256),
                    start=(kt == 0), stop=(kt == 1))
            nc.vector.tensor_add(
                vts[:, :, pb, 0:64],
                pvt[0:108, :].rearrange('p (h d) -> p h d', d=64),
                b3bc[:].rearrange('p (h d) -> p h d', d=64))

        # conv-branch masked copies early: DVE does them under attention
        for (src_, dst) in ((qs, qc), (ks, kc), (vs, vc)):
            for t in range(2):
                nc.vector.tensor_mul(dst[:, t], src_[:, t, 2:16, 2:52], mcs)

        # ---- attention (chunk-major QK, hh-interleaved row-halves) ----
        for t in range(2):
            attE_t = {}
            for ct in range(9):
                rb0 = max(0, ct - 3)
                rb1 = min(5, ct)
                n = rb1 - rb0 + 1
                for hh in range(2):
                    h = 2 * t + hh
                    hp = 64 * hh
                    patt = PS.tile([128, 384], F32, tag='ps')
                    nc.tensor.matmul(
                        patt[:, 0:96 * n],
                        lhsT=kms[hp:hp + 64, t, 108 * ct:108 * ct + 128],
                        rhs=qs[hp:hp + 64, t, 3 + 2 * rb0:3 + 2 * rb0 + 2 * n, 3:51],
                        start=True, stop=True)
                    aE = AE.tile([108, 384], F16, tag='attE')
                    attE_t[(hh, ct)] = aE
                    nc.scalar.activation(aE[:, 0:96 * n], patt[0:108, 0:96 * n],
                                         AF.Exp, scale=0.125)
                    # reversed mask: block b of a full chunk is cc = 3-b;
                    # partial chunks are contiguous slices of it
                    moff = 96 * (3 - ct) if ct < 3 else 0
                    nc.vector.tensor_mul(
                        aE[:, 0:96 * n],
                        aE[:, 0:96 * n],
                        v16(O_MAR + moff, 96 * n, 108))
                if ct >= 3:
                    rb = ct - 3
                    for hh in range(2):
                        h = 2 * t + hh
                        pav = PS.tile([96, 65], F32, tag='ps')
                        for cc in range(4):
                            cte = rb + cc
                            b = rb - max(0, cte - 3)
                            nc.tensor.matmul(
                                pav[:],
                                lhsT=attE_t[(hh, cte)][:, 96 * b:96 * b + 96],
                                rhs=vts[:, h, cte, :],
                                start=(cc == 0), stop=(cc == 3))
                        rcp = WK.tile([96, 1], F32, tag='rcp')
                        nc.vector.reciprocal(rcp[:], pav[:, 64:65])
                        nc.vector.tensor_scalar_mul(
                            opx[:, rb, 64 * h:64 * h + 64], pav[:, 0:64], rcp[:])

        # ---- conv branch (emitted after attention; runs late on PE) ----
        srcs = [(qc, 0), (qc, 1), (kc, 0), (kc, 1), (vc, 0), (vc, 1)]
        for ig in range(6):
            for chk in range(2):
                gm = 117 if GSIZE6[ig] == 13 else 54
                pf = PS.tile([128, 7, CGC], F32, tag='ps')
                for kti, (src_, t) in enumerate(srcs):
                    nc.tensor.matmul(
                        pf[0:gm, :],
                        lhsT=bdf_v[:, ig * 6 + kti, 0:gm],
                        rhs=src_[:, t, 7 * chk:7 * chk + 7, :],
                        start=(kti == 0), stop=(kti == 5))
                nc.scalar.activation(fs[0:gm, ig, 7 * chk:7 * chk + 7, :],
                                     pf[0:gm, :], AF.Copy)

        # depthwise: accumulate 3 groups x 9 shifts straight into the
        # final channel-tile layout (weight cols at absolute positions)
        # per-group aligned M windows within the 128-ch tile
        WIN = [(0, 64), (0, 128), (96, 32)]
        for tt in range(2):
            for hf in range(2):
                pcv = PS.tile([128, 6, 48], F32, tag='ps')
                first, last = 3 * tt, 3 * tt + 2
                for ig in range(first, last + 1):
                    w0, wn = WIN[ig - first]
                    gk = 117 if GSIZE6[ig] == 13 else 54
                    si = 0
                    for dr in (-1, 0, 1):
                        for dc in (-1, 0, 1):
                            st = (ig == first and si == 0)
                            en = (ig == last and si == 8)
                            a0, an = (0, 128) if (st or en) else (w0, wn)
                            nc.tensor.matmul(
                                pcv[a0:a0 + an, :, :],
                                lhsT=bdd_v[0:gk, ig * 9 + si, a0:a0 + an],
                                rhs=fs[0:gk, ig, 1 + 6 * hf + dr:7 + 6 * hf + dr,
                                       1 + dc:49 + dc],
                                start=st, stop=en,
                                tile_position=(0, a0) if a0 else None,
                                skip_group_check=not (st or en))
                            si += 1
                nc.scalar.activation(ocv[:, tt, 6 * hf:6 * hf + 6, :], pcv[:],
                                     AF.Copy)

        # ---- final combine (transpose att to ch-major, add conv) ----
        for cht in range(2):
            for rb in range(6):
                pfin = PS.tile([128, 96], F16, tag='ps')
                nc.tensor.transpose(
                    pfin[:], in_=opx[:, rb, 128 * cht:128 * cht + 128],
                    identity=id96)
                nc.vector.tensor_add(
                    osb[:, cht, 2 * rb:2 * rb + 2, :], pfin[:],
                    ocv[:, cht, 2 * rb:2 * rb + 2, :])
        for cht in range(2):
            nc.sync.dma_start(out=out_d[cht], in_=osb[:, cht])

    nc.finalize()
    return nc


def _host_prep(inputs):
    """Build per-core input maps (two packed blobs per core)."""
    x, y, z = inputs['x'], inputs['y'], inputs['z']
    W1, b1 = inputs['W1'], inputs['b1']
    W2, b2 = inputs['W2'], inputs['b2']
    W3, b3 = inputs['W3'], inputs['b3']
    Wp, bp = inputs['Wp'], inputs['bp']
    Wfc, Wdep = inputs['Wfc'], inputs['Wdep']

    f32, f16 = np.float32, np.float16

    def pad_rc(a):  # reflect-pad H and W by 3: [B, C, 54, 54]
        return np.pad(a, ((0, 0), (0, 0), (PAD, PAD), (PAD, PAD)), mode='reflect')

    xp, yp, zp = pad_rc(x), pad_rc(y), pad_rc(z)

    loc_w = np.broadcast_to(np.linspace(-1.0, 1.0, W, dtype=f32)[None, :], (H, W))
    loc_h = np.broadcast_to(np.linspace(-1.0, 1.0, H, dtype=f32)[:, None], (H, W))
    pos = np.stack([loc_w, loc_h], axis=0)
    posp = np.pad(pos, ((0, 0), (PAD, PAD), (PAD, PAD)), mode='reflect')

    # shared fp16 blob pieces
    shared16 = np.zeros((128, NB16 - O_W1), f16)   # from O_W1 onward
    def put(off, arr):                              # off relative to O_W1
        p, n = arr.shape
        shared16[0:p, off:off + n] = arr

    for wi, Wm in ((0, W1), (512, W2), (1024, W3)):
        wt = np.ascontiguousarray(Wm.T.astype(f16)).reshape(2, 128, 256)
        put(wi, np.ascontiguousarray(wt.transpose(1, 0, 2).reshape(128, 512)))
    put(O_WPT - O_W1, Wp.T.astype(f16))

    # reversed attention mask [108, 4, 96]: block b holds cc = 3-b
    ma = np.zeros((2, SLAB_C, 4, 2, W), f16)
    for wr in range(2):
        for cp in range(SLAB_C):
            for cc in range(4):
                for r2 in range(2):
                    if 0 <= 2 * cc + wr - r2 <= 6:
                        for c in range(W):
                            if 0 <= cp - c <= 6:
                                ma[wr, cp, cc, r2, c] = 1.0
    ma_r = ma[:, :, ::-1].reshape(108, 384)
    put(O_MAR - O_W1, np.ascontiguousarray(ma_r))

    # block-diagonal f weights [128, 36, 128] over 6 aligned g-groups
    GSTART = [0, 13, 26, 32, 45, 58]
    GSIZE = [13, 13, 6, 13, 13, 6]
    bdf = np.zeros((128, 36, 128), f16)
    for ig in range(6):
        gs, gn = GSTART[ig], GSIZE[ig]
        for kti in range(6):
            tau, tt = kti // 2, kti % 2
            for hp in range(2):
                cprime = 4 * tau + 2 * tt + hp
                for gl in range(gn):
                    g = gs + gl
                    for i in range(9):
                        bdf[64 * hp + g, ig * 6 + kti, gl * 9 + i] = Wfc[i, cprime]
    put(O_BDF - O_W1, bdf.reshape(128, 36 * 128))

    # block-diagonal depthwise weights [117, 54, 128]; M-cols sit at the
    # absolute within-tile channel position (0.5 folded in)
    bdd = np.zeros((MF, 54, 128), f16)
    for ig in range(6):
        gs, gn = GSTART[ig], GSIZE[ig]
        ttile = (4 * gs) // 128
        for si, (dr, dc) in enumerate([(a, b) for a in (-1, 0, 1) for b in (-1, 0, 1)]):
            kh, kw = dr + 1, dc + 1
            for gl in range(gn):
                g = gs + gl
                for i in range(9):
                    for op in range(4):
                        col = 4 * g + op - 128 * ttile
                        bdd[gl * 9 + i, ig * 9 + si, col] = \
                            0.5 * Wdep[4 * g + op, i, kh, kw]
    put(O_BDD - O_W1, bdd.reshape(MF, 54 * 128))

    put(O_ID - O_W1, np.eye(96, dtype=f16))

    # shared fp32 blob
    shared32 = np.zeros((128, NB32), f32)
    shared32[:, O_B1:O_B1 + 2] = b1.astype(f32).reshape(2, 128).T
    shared32[:, O_B2:O_B2 + 2] = b2.astype(f32).reshape(2, 128).T
    shared32[:, O_B3:O_B3 + 2] = b3.astype(f32).reshape(2, 128).T
    shared32[0:64, O_BP] = bp.astype(f32)
    shared32[0:108, O_B3BC:O_B3BC + 256] = b3.astype(f32)[None, :]

    in_maps = []
    for core in range(NCORES):
        bi, blk = core // 4, core % 4
        r0 = RPC * blk

        blob16 = np.zeros((128, NB16), f16)
        blob16[:, O_W1:] = shared16

        def put_slab(base, ap):
            s = ap[bi][:, r0:r0 + SLAB_R, :].astype(f16).reshape(256, PX)
            for t in range(2):
                blob16[:, base + PXP * t:base + PXP * t + PX] = \
                    s[128 * t:128 * t + 128]

        put_slab(O_XS, xp)
        put_slab(O_YS, yp)
        put_slab(O_ZS, zp)
        blob16[0:2, O_POS:O_POS + PX] = \
            posp[:, r0:r0 + SLAB_R, :].astype(f16).reshape(2, PX)

        mc = np.zeros((CGR, CGC), f16)
        for i in range(CGR):
            if 0 <= r0 - 1 + i < H:
                mc[i, 1:49] = 1.0
        blob16[:, O_MCS:O_MCS + CPX] = mc.reshape(1, CPX)

        in_maps.append({'blob16': blob16, 'blob32': shared32})
    return in_maps


def kernel(**inputs):
    global last_results
    from concourse.bass_utils import run_bass_kernel_spmd

    if os.environ.get('LDWOPT'):
        from concourse.compiler_utils import get_compiler_flags, set_compiler_flags
        flags = [f.replace('--enable-ldw-opt=false', '--enable-ldw-opt=true')
                 for f in get_compiler_flags()]
        set_compiler_flags(flags)

    if 'nc' not in _CACHE:
        _CACHE['nc'] = _build_nc()
    nc = _CACHE['nc']

    in_maps = _host_prep(inputs)
    trace = bool(os.environ.get('BASS_TRACE'))
    res = run_bass_kernel_spmd(nc, in_maps, list(range(NCORES)), trace=trace)
    last_results = res

    out = np.zeros((B, C, H, W), np.float32)
    for core in range(NCORES):
        bi, blk = core // 4, core % 4
        r0 = RPC * blk
        o = res.results[core]['out']           # [2, 128, 12, 48]
        out[bi, 0:128, r0:r0 + RPC, :] = o[0]
        out[bi, 128:256, r0:r0 + RPC, :] = o[1]
    return out

